# revision 42
# baseline (speedup 1.0000x reference)
"""Trainium2 Bass kernel for nn_AdaptiveTransformerModel (gated multi-head
self-attention with per-head scalar normalization), distributed over 8
NeuronCores via head parallelism + AllToAll.

One fused instruction stream, scalar-engine(exp)/PE co-bound steady state:
  - Host passes X pre-transposed ([D, B*T] bf16) so X^T tiles are plain
    contiguous DMAs.
  - Per-core computation (2 heads, all batches), bf16 matmuls / fp32 stats:
    Q^T/K^T/V^T = (X W + b).T laid out [128=(2 heads x 64 hd), B*T]. Per
    chunk (512 q): S^T = K Q^T as a row-tiled pair over the 2 heads
    (concurrent via PE row groups 0-1/2-3), E = exp(S^T/8) in one ACT pass
    over both heads [128, 1024], O^T accumulated as [V | ones].T E (row 64
    = softmax denominators).
  - QKV projections for batches 1..3, V transposes, the Wo load, drains,
    normalizes, stats and collectives are interleaved as deadline-tagged
    quanta into the attention k-loop; QKV bias adds and PSUM drains run on
    DVE, bulk SBUF elementwise on the Pool engine, so the scalar engine
    does exp only.
  - Rotated software pipeline: chunk cc's S(0)/exp(0) issue at the end of
    chunk cc-1 (before its drain) so ACT never idles at chunk boundaries.
    Drain = 2 PSUM->SBUF f32 copies (banks free in ~1.3us); sumsq matmuls
    are deferred quanta writing an aux PSUM bank.
  - 4 eager AllToAll pieces (4 chunks each, [8, 128, 256] quarter slots):
    piece p carries chunks 4p..4p+3; fired as soon as its chunks are
    softmax-normalized, so only the last piece's wire time is exposed.
  - Per-head scale s_h = 1/max(mean ||O_h||, 1e-5) approximated over
    chunks 0..12 (13/16 of rows, rel effect ~4e-4), computed via
    exp(0.5*ln(sumsq))*recip on the ACT natural_log_exp table (no table
    switch), AllGathered early, folded into the outgoing softmax recip for
    piece 3 and applied to received G for pieces 0-2 — so the final
    projections (piece pairs, N=512 moving) start immediately at stream
    end and cover the last A2A's latency.
"""
import os
import sys
from bisect import insort

import numpy as np

for _p in ("/root/.axon_site", "/root/.axon_site/_ro/trn_rl_repo", "/opt/trn_rl_repo"):
    if os.path.isdir(_p) and _p not in sys.path:
        sys.path.append(_p)

import ml_dtypes
import concourse.bass as bass
import concourse.bacc as bacc
import concourse.mybir as mybir
import concourse.tile as tile
from concourse import bass_utils
from concourse.bass import ts
from concourse.masks import make_identity

f32 = mybir.dt.float32
bf16 = mybir.dt.bfloat16

AF = mybir.ActivationFunctionType
ALU = mybir.AluOpType
BF16NP = ml_dtypes.bfloat16

# problem shapes (hardcoded per harness contract)
B, T, D, H = 4, 2048, 1024, 16
HD = 64
NCORES = 8


class Cfg:
    def __init__(self, B=B, T=T, D=D, ncores=NCORES):
        self.B, self.T, self.D, self.ncores = B, T, D, ncores
        self.RT = B * T                  # flattened rows
        self.RSLC = self.RT // ncores    # output row slice per core
        self.DCH = D // 128              # contraction chunks for D
        self.TQ = 512                    # q-chunk width
        self.NQC = T // self.TQ          # q-chunks per batch
        self.NKT = T // 128              # k-tiles per batch
        self.NCH = self.B * self.NQC     # total q-chunks (16)
        self.NPIECE = 4                  # A2A pieces
        self.CPP = self.NCH // self.NPIECE  # chunks per piece (4)
        self.QH = 256                    # quarter-chunk a2a slot width
        self.NSTAT = 16                  # chunks used for the head-norm mean
        assert T % self.TQ == 0 and D % 128 == 0 and self.RT % ncores == 0


def build_body(ctx, tc, cfg, x, wq, wk, wv, bq, bk, bv, wo, bo, out,
               dbg=None):
    from contextlib import ExitStack
    nc = tc.nc
    DCH, TQ, NKT, NQC = cfg.DCH, cfg.TQ, cfg.NKT, cfg.NQC
    NCH, CPP, QH = cfg.NCH, cfg.CPP, cfg.QH
    RW = TQ                              # QKV row-chunk width
    RCPB = cfg.T // RW                   # row-chunks per batch (4)
    HCH = (128 * cfg.ncores) // 128      # final contraction chunks (8)

    constp = ctx.enter_context(tc.tile_pool(name="const", bufs=1))
    ident = constp.tile([128, 128], f32)
    make_identity(nc, ident[:])
    ident_bf = constp.tile([128, 128], bf16)
    nc.vector.tensor_copy(ident_bf[:], ident[:])
    ones_f32 = constp.tile([128, 1], f32)
    nc.vector.memset(ones_f32[:], 1.0)
    ones_bf = constp.tile([128, 1], bf16)
    nc.vector.tensor_copy(ones_bf[:], ones_f32[:])
    warm_bf = constp.tile([128, 512], bf16)
    nc.vector.memset(warm_bf[:, 0:16], 0.25)

    persistp = ctx.enter_context(tc.tile_pool(name="persist", bufs=1))
    qt_all = persistp.tile([128, cfg.RT], bf16, name="qt_all")
    kt_all = persistp.tile([128, cfg.RT], bf16, name="kt_all")
    vaug = persistp.tile([128, cfg.B, NKT, 2, 65], bf16, name="vaug")
    wo_sb = persistp.tile([128, HCH, cfg.D], bf16, name="wo_sb")
    g_all = persistp.tile([128, HCH, cfg.NPIECE, QH], bf16, name="g_all")
    bq_sb = constp.tile([128, 1], f32)
    bk_sb = constp.tile([128, 1], f32)
    bv_sb = constp.tile([128, 1], f32)
    bo_sb = constp.tile([128, DCH], f32)

    wqkvp = ctx.enter_context(tc.tile_pool(name="wqkv", bufs=1))
    wq_sb = wqkvp.tile([128, DCH, 128], bf16)
    wk_sb = wqkvp.tile([128, DCH, 128], bf16)
    wv_sb = wqkvp.tile([128, DCH, 128], bf16)

    dramp = ctx.enter_context(tc.tile_pool(name="dram", bufs=1, space="DRAM"))
    # per-chunk stat rows: (head, kind[0=softmax-recip, 1=sumsq], cc, q)
    sr_dram = dramp.tile([2, 2, NCH, TQ], f32)
    a2a_in = [dramp.tile([cfg.ncores, 128, QH], bf16, name=f"a2a_in{p}")
              for p in range(cfg.NPIECE)]
    a2a_out = [dramp.tile([cfg.ncores, 128, QH], bf16, name=f"a2a_out{p}")
               for p in range(cfg.NPIECE)]
    s_dram = dramp.tile([1, 2], f32, name="s_scratch")
    s_ag = dramp.tile([cfg.ncores, 2], f32, name="s_ag")

    # pools
    epool = ctx.enter_context(tc.tile_pool(name="epool", bufs=3))
    ph2 = ctx.enter_context(tc.tile_pool(name="ph2", bufs=2))
    xtp = ctx.enter_context(tc.tile_pool(name="xt", bufs=5))
    vtp = ctx.enter_context(tc.tile_pool(name="vtmp", bufs=2))
    scrp = ctx.enter_context(tc.tile_pool(name="scr", bufs=4))
    stgp = ctx.enter_context(tc.tile_pool(name="stg", bufs=3))
    rcstp = ctx.enter_context(tc.tile_pool(name="rcst", bufs=5))

    mainps = ExitStack()
    sps = mainps.enter_context(tc.tile_pool(name="sps", bufs=2, space="PSUM"))
    ops = mainps.enter_context(tc.tile_pool(name="ops", bufs=1, space="PSUM"))
    auxps = mainps.enter_context(tc.tile_pool(name="auxps", bufs=2, space="PSUM"))

    # ---------------- warm-up + ACT table preload ----------------
    # ~14 dummy matmuls flip the PE HAM clock gate to 8/8 during the DMA
    # head; tiny exp+ln pulls the natural_log_exp table load off the stream.
    wps = auxps.tile([128, 512], f32, tag="qkv", bufs=1, name="wps")
    for _ in range(7):
        nc.tensor.matmul(wps[:], ident_bf[:], warm_bf[:], start=True,
                         stop=True, skip_group_check=True)
    wact = constp.tile([1, 8], f32)
    nc.scalar.activation(wact[:], warm_bf[0:1, 0:8], AF.Exp)
    nc.scalar.activation(wact[:], warm_bf[0:1, 0:8], AF.Ln)

    # ---------------- initial DMAs (critical-path first) ----------------
    nc.sync.dma_start(wk_sb[:], wk.rearrange("(c p) m -> p c m", p=128))

    # ---------------- QKV quanta ----------------
    xt_tiles, qkv_ps, vt_tiles = {}, {}, {}

    def u_xt(rc):
        def f():
            xt = xtp.tile([128, DCH, RW], bf16, tag="xt", name=f"xt{rc}")
            xt_tiles[rc] = xt
            for d in range(DCH):
                nc.sync.dma_start(xt[:, d, :], x[ts(d, 128), ts(rc, RW)])
        return f

    def u_proj_mm(rc, w_sb, d0, d1, proj):
        def f():
            if d0 == 0:
                qkv_ps[(rc, proj)] = auxps.tile(
                    [128, RW], f32, tag="qkv", bufs=1, name=f"ps_{proj}{rc}")
            ps = qkv_ps[(rc, proj)]
            xt = xt_tiles[rc]
            for d in range(d0, d1):
                nc.tensor.matmul(ps[:], w_sb[:, d, :], xt[:, d, :],
                                 start=(d == 0), stop=(d == DCH - 1),
                                 skip_group_check=True)
        return f

    def u_proj_bias(rc, b_sb, dest, proj):
        def f():
            ps = qkv_ps.pop((rc, proj))
            nc.vector.tensor_scalar(out=dest[:, ts(rc, RW)], in0=ps[:],
                                    scalar1=b_sb[:, 0:1], scalar2=None,
                                    op0=ALU.add)
        return f

    def u_v_bias(rc):
        def f():
            ps = qkv_ps.pop((rc, "v"))
            vt = vtp.tile([128, RW], bf16, tag="vt", name=f"vt{rc}")
            vt_tiles[rc] = vt
            nc.vector.tensor_scalar(out=vt[:], in0=ps[:],
                                    scalar1=bv_sb[:, 0:1], scalar2=None,
                                    op0=ALU.add)
        return f

    def u_v_tr(rc, j):
        def f():
            vt = vt_tiles[rc]
            b_idx = (rc * RW) // cfg.T
            kt_idx = ((rc * RW) % cfg.T) // 128 + j
            vp = auxps.tile([128, 2, 64], bf16, tag="vp", bufs=1,
                            name=f"vp{rc}_{j}")
            nc.tensor.transpose(
                vp[:].rearrange("p h c -> p (h c)"), vt[:, ts(j, 128)],
                ident_bf[:])
            nc.vector.tensor_copy(vaug[:, b_idx, kt_idx, :, 0:64], vp[:])
        return f

    def u_wo_load():
        def f():
            nc.sync.dma_start(wo_sb[:], wo.rearrange("(c p) m -> p c m", p=128))
        return f

    def qkv_units_for_rc(rc, proj):
        w_sb, b_sb, dest = {
            "k": (wk_sb, bk_sb, kt_all), "q": (wq_sb, bq_sb, qt_all),
            "v": (wv_sb, bv_sb, None)}[proj]
        us = []
        for d0 in range(0, DCH, 4):
            us.append(u_proj_mm(rc, w_sb, d0, d0 + 4, proj))
        if proj == "v":
            us.append(u_v_bias(rc))
            us += [u_v_tr(rc, j) for j in range(RW // 128)]
        else:
            us.append(u_proj_bias(rc, b_sb, dest, proj))
        return us

    # pending: (deadline_cc, seq, unit) quanta run inside the k-loop.
    pending = []
    _seq = [0]

    def add_u(dl, u):
        insort(pending, (dl, _seq[0], u))
        _seq[0] += 1

    for i in range(2, RCPB):  # batch 0's later K/V (needed mid-chunk-0)
        dl = (4 * i - 2) / NKT
        for u in qkv_units_for_rc(i, "k"):
            add_u(dl, u)
        for u in qkv_units_for_rc(i, "v"):
            add_u(dl, u)
    for i in range(1, RCPB):  # batch 0's deferred Q projections (chunk i)
        for u in qkv_units_for_rc(i, "q"):
            add_u(i - 0.6, u)
    for b in range(1, cfg.B):
        dl = b * NQC
        rcs = [b * RCPB + i for i in range(RCPB)]
        rest = {rc: (qkv_units_for_rc(rc, "k") + qkv_units_for_rc(rc, "v")
                     + qkv_units_for_rc(rc, "q")) for rc in rcs}
        merged = [u_xt(rcs[0]), u_xt(rcs[1])]
        for i, rc in enumerate(rcs):
            merged += rest[rc]
            if i + 2 < RCPB:
                merged.append(u_xt(rcs[i + 2]))
        for k, u in enumerate(merged):
            add_u(dl - 1.99 + 1.9 * k / len(merged), u)
        if b == 2:
            add_u(dl - 0.5, u_wo_load())

    # ---------------- stats / normalize / a2a units ----------------
    scratch_tiles = {}     # cc -> [scr_h0, scr_h1]  ([65, TQ] f32)
    rcst_tiles = {}        # cc -> [33, TQ] f32 sumsq row at partition hl*32
    rc8_tiles = {}         # cc -> [64, 2, 8] f32 softmax recips (repacked)
    s_sb = constp.tile([1, 2], f32)
    svec_sb = constp.tile([128, HCH], f32)

    def u_drain2(cc, eng=None):
        # sumsq via ones-matmul into aux PSUM; stash row + flush the
        # chunk's [recip, sumsq] rows to DRAM. Only for chunks feeding s.
        def f():
            de = eng or nc.sync
            scr = scratch_tiles[cc]
            rcst = rcst_tiles[cc]
            sq = ph2.tile([128, TQ], bf16, tag="sq", name="sq")
            for hl in range(2):
                hs = slice(hl * 64, (hl + 1) * 64)
                nc.vector.tensor_tensor(out=sq[hs, :], in0=scr[hl][0:64, :],
                                        in1=scr[hl][0:64, :], op=ALU.mult)
            ss = auxps.tile([128, TQ], f32, tag="vp", bufs=1, name=f"ss{cc}")
            for hl in range(2):
                hs = slice(hl * 64, (hl + 1) * 64)
                nc.tensor.matmul(ss[ts(hl, 64), :][0:1, :], ones_bf[hs, 0:1],
                                 sq[hs, :], start=True, stop=True,
                                 skip_group_check=True,
                                 tile_position=(hl * 64, hl * 64))
            for hl in range(2):
                nc.vector.tensor_copy(rcst[hl * 32:hl * 32 + 1, :],
                                      ss[hl * 64:hl * 64 + 1, :])
                de.dma_start(sr_dram[hl, 1, cc, :][None, :],
                             rcst[hl * 32:hl * 32 + 1, :])
        return f

    def u_stats_all(eng=None):
        # one-shot head-norm stats over chunks 0..NSTAT-1:
        # mean ||O_norm|| = mean over rows of exp(0.5*ln(sumsq)) * recip,
        # packed [64, 2, 96] per head (12 chunks x 512 = 64 x 96).
        def f():
            de = eng or nc.sync
            ncol = cfg.NSTAT * TQ // 64
            stp = ph2.tile([64, 2, ncol], f32, tag="stp", bufs=1, name="stp")
            rcp = ph2.tile([64, 2, ncol], f32, tag="rcp", bufs=1, name="rcp")
            for hl in range(2):
                de.dma_start(
                    stp[:, hl, :],
                    sr_dram[hl, 1, 0:cfg.NSTAT, :].rearrange("c q -> (c q)")
                    .rearrange("(p n) -> p n", p=64))
                de.dma_start(
                    rcp[:, hl, :],
                    sr_dram[hl, 0, 0:cfg.NSTAT, :].rearrange("c q -> (c q)")
                    .rearrange("(p n) -> p n", p=64))
            lnt = ph2.tile([64, 2, ncol], f32, tag="lnt", bufs=1,
                           name="lnt")
            nc.scalar.activation(lnt[:], stp[:], AF.Ln)
            nc.scalar.activation(stp[:], lnt[:], AF.Exp, scale=0.5)
            nc.vector.tensor_tensor(out=stp[:], in0=stp[:], in1=rcp[:],
                                    op=ALU.mult)
            red = ph2.tile([64, 2], f32, tag="red", bufs=1, name="red")
            for hl in range(2):
                nc.vector.tensor_reduce(red[:, hl:hl + 1], stp[:, hl, :],
                                        axis=mybir.AxisListType.X, op=ALU.add)
            ntot = auxps.tile([1, 2], f32, tag="vp", bufs=1, name="ntot")
            nc.tensor.matmul(ntot[:], ones_f32[0:64, 0:1], red[:],
                             start=True, stop=True, skip_group_check=True)
            nrows = cfg.NSTAT * TQ
            nc.vector.tensor_scalar(out=s_sb[:], in0=ntot[:],
                                    scalar1=1.0 / nrows, scalar2=1e-5,
                                    op0=ALU.mult, op1=ALU.max)
            nc.vector.reciprocal(s_sb[:], s_sb[:])
            (eng or nc.sync).dma_start(s_dram[:], s_sb[:])
        return f

    def u_s_ag():
        def f():
            nc.gpsimd.collective_compute(
                "AllGather", ALU.bypass,
                replica_groups=[list(range(cfg.ncores))],
                ins=[s_dram[:].opt()], outs=[s_ag[:].opt()])
        return f

    def u_svec_load():
        def f():
            for hl in range(2):
                nc.gpsimd.dma_start(
                    svec_sb[hl * 64:(hl + 1) * 64, :],
                    s_ag[:, hl][None, :].to_broadcast((64, HCH)))
        return f

    def u_normalize(cc, eng=None):
        # multiply scratch by softmax recip rows, stage into a2a_in
        def f():
            de = eng or nc.sync
            p, j = divmod(cc, CPP)
            scr = scratch_tiles.pop(cc)
            rb = stgp.tile([64, 2, TQ], f32, tag="rb", name="rb")
            for hl in range(2):
                de.dma_start(
                    rb[:, hl, :],
                    sr_dram[hl, 0, cc, :][None, :].to_broadcast((64, TQ)))
            st = stgp.tile([128, TQ], bf16, tag="stage", name="stage")
            for hl in range(2):
                hs = slice(hl * 64, (hl + 1) * 64)
                nc.vector.tensor_tensor(out=st[hs, :], in0=scr[hl][0:64, :],
                                        in1=rb[:, hl, :], op=ALU.mult)
            de.dma_start(
                a2a_in[p][2 * j:2 * j + 2, :, :].rearrange("s p q -> p s q"),
                st[:].rearrange("p (s q) -> p s q", s=2))
            if dbg is not None:
                nc.sync.dma_start(dbg["st"][cc], st[:])
                nc.sync.dma_start(dbg["scr"][cc],
                                  scr[0][0:64, :])
                nc.sync.dma_start(dbg["rb"][cc], rb[:, 0, :])
        return f

    def u_fire_piece(p):
        def f():
            if dbg is not None:
                nc.sync.dma_start(dbg["pre_in"][p], a2a_in[p][:])
            nc.gpsimd.collective_compute(
                "AllToAll", ALU.bypass,
                replica_groups=[list(range(cfg.ncores))],
                ins=[a2a_in[p][:].opt()], outs=[a2a_out[p][:].opt()])
        return f

    def u_gload(p, eng=None):
        # deferred so its collective-completion wait doesn't park the
        # gpsimd queue right after the fire (Pool compute shares the
        # queue); split per source core to spread descriptors across DMA
        # engines (a single transposing DMA is descriptor-bound, ~20us)
        def f():
            e = eng or nc.gpsimd
            for c in range(cfg.ncores):
                e.dma_start(g_all[:, c, p, :], a2a_out[p][c])
        return f

    def gscale(j, p0, p1):
        nc.vector.tensor_scalar(out=g_all[:, j, p0:p1, :],
                                in0=g_all[:, j, p0:p1, :],
                                scalar1=svec_sb[:, j:j + 1],
                                scalar2=None, op0=ALU.mult)

    for cc in range(15):
        add_u(cc + 1.25, u_drain2(cc))
    for cc in range(15):
        add_u(cc + 1.5, u_normalize(cc))
    for p in range(3):
        add_u(4 * p + 4.7, u_fire_piece(p))
        add_u(4 * p + 6.2 if p < 2 else 14.2, u_gload(p))

    # ------------- head: QKV rc0/rc1 K+V and chunk-0's Q ------------------
    u_xt(0)()
    nc.sync.dma_start(wv_sb[:], wv.rearrange("(c p) m -> p c m", p=128))
    nc.sync.dma_start(wq_sb[:], wq.rearrange("(c p) m -> p c m", p=128))
    nc.sync.dma_start(bq_sb[:], bq[:, None])
    nc.sync.dma_start(bk_sb[:], bk[:, None])
    nc.sync.dma_start(bv_sb[:], bv[:, None])
    nc.sync.dma_start(bo_sb[:], bo.rearrange("(c p) -> p c", p=128))
    for _ in range(7):
        nc.tensor.matmul(wps[:], ident_bf[:], warm_bf[:], start=True,
                         stop=True, skip_group_check=True)
    nc.vector.tensor_copy(
        vaug[:, :, :, :, 64:65],
        ones_f32[:].to_broadcast((128, cfg.B, NKT, 2, 1)))
    for rc in range(1, RCPB):
        u_xt(rc)()
    for rc in range(2):
        for u in qkv_units_for_rc(rc, "k"):
            u()
        for u in qkv_units_for_rc(rc, "v"):
            u()
    for u in qkv_units_for_rc(0, "q"):
        u()

    # ---------------- fused attention stream ----------------
    def mk_flush(b, o_ps):
        def flush_o(te, e_tile):
            for hl in range(2):
                nc.tensor.matmul(o_ps[hl][0:65, :], vaug[:, b, te, hl, :],
                                 e_tile[:, ts(hl, TQ)],
                                 start=(te == 0), stop=(te == NKT - 1),
                                 skip_group_check=True)
        return flush_o

    def s_exp(cc, t):
        b = cc // NQC
        k0 = b * cfg.T + t * 128
        c0 = cc * TQ
        s_pair = sps.tile([128, 2 * TQ], f32, tag="s", name="s_pair")
        for hl in range(2):
            hs = slice(hl * 64, (hl + 1) * 64)
            nc.tensor.matmul(s_pair[:, ts(hl, TQ)],
                             kt_all[hs, k0:k0 + 128],
                             qt_all[hs, c0:c0 + TQ],
                             start=True, stop=True)
        e_pair = epool.tile([128, 2 * TQ], bf16, tag="e", name="e_pair")
        nc.scalar.activation(e_pair[:], s_pair[:], AF.Exp, scale=0.125)
        return e_pair

    def run_pending(limit_dl):
        while pending and pending[0][0] <= limit_dl:
            pending.pop(0)[2]()

    def preamble(cc):
        run_pending(cc)
        o_ps = [ops.tile([128, TQ], f32, tag=f"o{hl}", name=f"o_ps{hl}")
                for hl in range(2)]
        return o_ps, s_exp(cc, 0)

    def drain(cc, o_ps):
        # scratch <- [O ; softmax-sum] per head (f32, frees PSUM fast),
        # then per-row softmax reciprocals into the group's rec tile
        scr = [scrp.tile([65, TQ], f32, tag=f"scr{hl}", name=f"scr{cc}_{hl}")
               for hl in range(2)]
        for hl in range(2):
            nc.vector.tensor_copy(scr[hl][:], o_ps[hl][0:65, :])
        scratch_tiles[cc] = scr
        rcst = rcstp.tile([33, TQ], f32, tag="rcst", name=f"rcst{cc}")
        rcst_tiles[cc] = rcst
        sm8 = rcstp.tile([64, 2, 8], f32, tag="sm8", name=f"sm8{cc}")
        rc8 = rcstp.tile([64, 2, 8], f32, tag="rc8", name=f"rc8{cc}")
        rc8_tiles[cc] = rc8
        for hl in range(2):
            nc.sync.dma_start(sm8[:, hl, :], scr[hl][64:65, :])
        nc.vector.reciprocal(rc8[:], sm8[:])
        for hl in range(2):
            nc.sync.dma_start(sr_dram[hl, 0, cc, :], rc8[:, hl, :])

    o_ps, prev_e = preamble(0)
    for cc in range(cfg.NCH):
        b = cc // NQC
        flush_o = mk_flush(b, o_ps)
        for t in range(1, NKT):
            run_pending(cc + t / NKT)
            e_next = s_exp(cc, t)
            flush_o(t - 1, prev_e)
            prev_e = e_next
            # interleave quanta under the exp-saturated ACT; only pop
            # units whose prerequisite emissions (dl - 1) have passed
            if pending:
                need = 0
                for dl, _s, _u in pending:
                    if dl <= cc + 2:
                        need += 1
                    else:
                        break
                slots = 2 * NKT - t
                budget = max(1, -(-need // slots)) if need else \
                    (1 if t % 2 == 0 else 0)
                for _ in range(budget):
                    if pending and pending[0][0] <= cc + 1:
                        pending.pop(0)[2]()
        flush_o(NKT - 1, prev_e)
        old_o_ps = o_ps
        if cc + 1 < cfg.NCH:
            o_ps, prev_e = preamble(cc + 1)
        drain(cc, old_o_ps)

    # ---------------- tail ----------------
    while pending:
        pending.pop(0)[2]()
    u_drain2(15, eng=nc.scalar)()
    u_normalize(15, eng=nc.scalar)()
    u_fire_piece(3)()
    u_stats_all(eng=nc.scalar)()
    u_s_ag()()
    u_svec_load()()
    u_gload(3, eng=nc.sync)()
    for j in range(HCH):
        gscale(j, 0, 4)

    if dbg is not None:
        for p in range(cfg.NPIECE):
            nc.sync.dma_start(dbg["a2a_in"][p], a2a_in[p][:])
            nc.sync.dma_start(dbg["a2a_out"][p], a2a_out[p][:])
        nc.sync.dma_start(dbg["sr"], sr_dram[:])
        nc.sync.dma_start(dbg["sag"], s_ag[:])
        nc.sync.dma_start(dbg["g"], g_all[:])

    # final projection: dsub-outer on the sps accumulator ring (a PSUM
    # pool transition here would barrier on collective-ring quiesce ~20us)
    with tc.tile_pool(name="pout", bufs=2) as poutp:
        for half in range(2):
            for dsub in range(DCH):
                ps = sps.tile([128, 2 * QH], f32, tag="s", name="pp")
                for j in range(HCH):
                    nc.tensor.matmul(ps[:], wo_sb[:, j, ts(dsub, 128)],
                                     g_all[:, j, 2 * half:2 * half + 2, :],
                                     start=(j == 0), stop=(j == HCH - 1))
                po = poutp.tile([128, 2 * QH], f32, tag="po", name="po")
                nc.vector.tensor_scalar(out=po[:], in0=ps[:],
                                        scalar1=bo_sb[:, dsub:dsub + 1],
                                        scalar2=None, op0=ALU.add)
                nc.sync.dma_start(
                    out[ts(dsub, 128), half * 2 * QH:(half + 1) * 2 * QH],
                    po[:])
    mainps.close()


def build_nc(cfg, compile=True, debug_outs=False):
    nc = bacc.Bacc("TRN2", target_bir_lowering=False, debug=False,
                   enable_asserts=False, num_devices=cfg.ncores)
    x = nc.dram_tensor("x", [cfg.D, cfg.RT], bf16, kind="ExternalInput").ap()
    wq = nc.dram_tensor("wq", [cfg.D, 128], bf16, kind="ExternalInput").ap()
    wk = nc.dram_tensor("wk", [cfg.D, 128], bf16, kind="ExternalInput").ap()
    wv = nc.dram_tensor("wv", [cfg.D, 128], bf16, kind="ExternalInput").ap()
    bq = nc.dram_tensor("bq", [128], f32, kind="ExternalInput").ap()
    bk = nc.dram_tensor("bk", [128], f32, kind="ExternalInput").ap()
    bv = nc.dram_tensor("bv", [128], f32, kind="ExternalInput").ap()
    wo = nc.dram_tensor("wo", [128 * cfg.ncores, cfg.D], bf16,
                        kind="ExternalInput").ap()
    bo = nc.dram_tensor("bo", [cfg.D], f32, kind="ExternalInput").ap()
    out = nc.dram_tensor("out", [cfg.D, cfg.RSLC], f32,
                         kind="ExternalOutput").ap()
    dbg = None
    if debug_outs:
        dbg = {
            "a2a_in": [nc.dram_tensor(f"dbg_a2a_in{p}",
                                      [cfg.ncores, 128, cfg.QH], bf16,
                                      kind="ExternalOutput").ap()
                       for p in range(cfg.NPIECE)],
            "a2a_out": [nc.dram_tensor(f"dbg_a2a_out{p}",
                                       [cfg.ncores, 128, cfg.QH], bf16,
                                       kind="ExternalOutput").ap()
                        for p in range(cfg.NPIECE)],
            "sr": nc.dram_tensor("dbg_sr", [2, 2, cfg.NCH, cfg.TQ], f32,
                                 kind="ExternalOutput").ap(),
            "sag": nc.dram_tensor("dbg_sag", [cfg.ncores, 2], f32,
                                  kind="ExternalOutput").ap(),
            "g": nc.dram_tensor("dbg_g", [128, 8, cfg.NPIECE, cfg.QH], bf16,
                                kind="ExternalOutput").ap(),
            "st": nc.dram_tensor("dbg_st", [cfg.NCH, 128, cfg.TQ], bf16,
                                 kind="ExternalOutput").ap(),
            "scr": nc.dram_tensor("dbg_scr", [cfg.NCH, 64, cfg.TQ], f32,
                                  kind="ExternalOutput").ap(),
            "rb": nc.dram_tensor("dbg_rb", [cfg.NCH, 64, cfg.TQ], f32,
                                 kind="ExternalOutput").ap(),
            "pre_in": [nc.dram_tensor(f"dbg_pre_in{p}",
                                      [cfg.ncores, 128, cfg.QH], bf16,
                                      kind="ExternalOutput").ap()
                       for p in range(cfg.NPIECE)],
        }
    from contextlib import ExitStack
    with tile.TileContext(nc) as tc, ExitStack() as ctx:
        build_body(ctx, tc, cfg, x, wq, wk, wv, bq, bk, bv, wo, bo, out,
                   dbg=dbg)
    if compile:
        nc.compile()
    return nc


def make_in_maps(cfg, inputs, H_total=None):
    """Host-side sharding: per-core input dicts."""
    H_tot = H_total or (2 * cfg.ncores)
    X = np.ascontiguousarray(
        np.asarray(inputs["hidden_states"], np.float32).reshape(cfg.RT, cfg.D).T
    ).astype(BF16NP)
    gate_clip = np.clip(np.asarray(inputs["gate"], np.float32), 0.0, 1.0)
    Wo = np.asarray(inputs["Wo"], np.float32)
    bo = np.asarray(inputs["bo"], np.float32)
    wo_all = np.ascontiguousarray(np.concatenate(
        [Wo[h] * (gate_clip[h] / H_tot) for h in range(H_tot)],
        axis=0)).astype(BF16NP)
    bo_sum = (bo * (gate_clip[:, None] / H_tot)).sum(axis=0).astype(np.float32)
    in_maps = []
    for c in range(cfg.ncores):
        h0, h1 = 2 * c, 2 * c + 1
        m = {
            "x": X,
            "wq": np.concatenate([inputs["Wq"][h0], inputs["Wq"][h1]], axis=1,
                                 dtype=np.float32).astype(BF16NP),
            "wk": np.concatenate([inputs["Wk"][h0], inputs["Wk"][h1]], axis=1,
                                 dtype=np.float32).astype(BF16NP),
            "wv": np.concatenate([inputs["Wv"][h0], inputs["Wv"][h1]], axis=1,
                                 dtype=np.float32).astype(BF16NP),
            "bq": np.concatenate([inputs["bq"][h0], inputs["bq"][h1]],
                                 dtype=np.float32),
            "bk": np.concatenate([inputs["bk"][h0], inputs["bk"][h1]],
                                 dtype=np.float32),
            "bv": np.concatenate([inputs["bv"][h0], inputs["bv"][h1]],
                                 dtype=np.float32),
            "wo": wo_all,
            "bo": bo_sum,
        }
        in_maps.append(m)
    return in_maps


def gather_out(cfg, results):
    """results: list of per-core out_maps -> full [B, T, D].

    Core c's out col block [p*256:(p+1)*256] = chunk (4p + c//2),
    column half (c%2).
    """
    QH = cfg.QH
    full = np.empty((cfg.RT, cfg.D), np.float32)
    for c, r in enumerate(results):
        o = np.asarray(r["out"])  # [D, RSLC]
        for p in range(cfg.NPIECE):
            chunk = 4 * p + c // 2
            r0 = chunk * cfg.TQ + (c % 2) * QH
            full[r0:r0 + QH] = o[:, p * QH:(p + 1) * QH].T
    return full.reshape(cfg.B, cfg.T, cfg.D)


_COMPILED = {}


def kernel(**inputs) -> np.ndarray:
    cfg = Cfg()
    key = "full"
    if key not in _COMPILED:
        _COMPILED[key] = build_nc(cfg)
    nc = _COMPILED[key]
    in_maps = make_in_maps(cfg, inputs)
    last_exc = None
    for _attempt in range(3):
        try:
            res = bass_utils.run_bass_kernel_spmd(
                nc, in_maps, core_ids=list(range(cfg.ncores)))
            return gather_out(cfg, res.results)
        except Exception as e:  # transient NRT_EXEC_UNIT_UNRECOVERABLE faults
            last_exc = e
    raise last_exc


if __name__ == "__main__":
    import reference
    inputs = {k: np.asarray(v) for k, v in reference.setup_inputs().items()}
    out = kernel(**inputs)
    exp = np.asarray(reference.reference(**inputs))
    rel = np.linalg.norm(out - exp) / np.linalg.norm(exp)
    print("Relative error:", rel)
